# revision 21
# baseline (speedup 1.0000x reference)
"""TLSTM (time-aware LSTM) scan + gather + MLP head for Trainium2, 8-core data parallel.

Model (per reference):
  per step t:  g = 1/log(e+t);  cs = tanh(c@Wd+bd);  c_adj = c + cs*(g-1)
               z = x_t@W + h@U + b;  i,f,cand,o = split(z); sig/sig/tanh/sig
               c = f*c_adj + i*cand;  h = o*tanh(c)
  out = sigmoid(gelu(h[pos]@W1+b1)@W2+b2)

Device mapping (per core, B_loc=16 of B=128):
  All state kept transposed: [units=128 partitions, batch=16 free].
  Per step one PSUM tile [128, 80]: cols [Si|Sf|So|CD|CS] (16 each).
  x@W contributions are issued as matmuls a step ahead (x pre-transposed &
  bf16-cast on host); U@h' matmuls accumulate on top on the critical path.
  All-tanh trick: sigmoid(z) = (tanh(z/2)+1)/2 with the 1/2 folded into W/U
  columns, carried state scaled c'=2c, h'=2h (folded into U and W1).
  Elementwise uses fused scalar_tensor_tensor ops:
     a1=(Sf+1)*c_adj', a2=(Si+1)*CD, c'_new=0.5*a1+a2, h'=(So+1)*tc
  Gather-at-position done arithmetically: sel = reduce_t(hist * onehot).
"""

import sys
import numpy as np

if "/opt/trn_rl_repo" not in sys.path:
    sys.path.insert(0, "/opt/trn_rl_repo")

import ml_dtypes

BF16 = ml_dtypes.bfloat16

B, T, D = 128, 1024, 256
UNITS, HID, OUT = 128, 64, 8
NCORES = 8
BL = B // NCORES  # 16 per-core batch


def build_module(Tn=T, slow_bias=False, debug_hist=False, sim_gelu=False, probe=(), groups=1):
    from contextlib import ExitStack

    import concourse.bass as bass
    import concourse.tile as tile
    from concourse import mybir
    from concourse.bacc import Bacc

    f32 = mybir.dt.float32
    bf16 = mybir.dt.bfloat16
    AF = mybir.ActivationFunctionType
    OPA = mybir.AluOpType

    nc = Bacc("TRN2", target_bir_lowering=False, debug=False, num_devices=NCORES)

    xT_d = nc.dram_tensor("xT", [D, BL * Tn], bf16, kind="ExternalInput")
    gm1_d = nc.dram_tensor("gm1", [128, Tn, BL], bf16, kind="ExternalInput")
    oh_d = nc.dram_tensor("oh", [128, Tn, BL], bf16, kind="ExternalInput")
    Wp_d = nc.dram_tensor("Wp", [D, 4 * UNITS], bf16, kind="ExternalInput")
    Up_d = nc.dram_tensor("Up", [UNITS, 4 * UNITS], bf16, kind="ExternalInput")
    Wd_d = nc.dram_tensor("Wdp", [UNITS, UNITS], bf16, kind="ExternalInput")
    W1_d = nc.dram_tensor("W1p", [UNITS, HID], bf16, kind="ExternalInput")
    W2_d = nc.dram_tensor("W2p", [HID, OUT], bf16, kind="ExternalInput")
    b1_d = nc.dram_tensor("b1v", [HID, 1], f32, kind="ExternalInput")
    b2_d = nc.dram_tensor("b2v", [OUT, 1], f32, kind="ExternalInput")
    if slow_bias:
        bias5_d = nc.dram_tensor("bias5", [5, UNITS], f32, kind="ExternalInput")
        sel5_d = nc.dram_tensor("sel5", [5, 80], f32, kind="ExternalInput")
    out_d = nc.dram_tensor("outT", [OUT, BL], f32, kind="ExternalOutput")
    histo_d = (
        nc.dram_tensor("histo", [128, Tn * BL], f32, kind="ExternalOutput")
        if debug_hist
        else None
    )
    if debug_hist:
        xto_d = nc.dram_tensor("xto", [128, BL * Tn], f32, kind="ExternalOutput")
        wo_d = nc.dram_tensor("wo", [128, 512], f32, kind="ExternalOutput")
        s0_d = nc.dram_tensor("s0", [128, 64], f32, kind="ExternalOutput")
        ps0_d = nc.dram_tensor("ps0", [128, 80], f32, kind="ExternalOutput")

    with tile.TileContext(nc) as tc, ExitStack() as ctx:
        singles = ctx.enter_context(tc.tile_pool(name="singles", bufs=1))
        tmp = ctx.enter_context(tc.tile_pool(name="tmp", bufs=3))
        cpool = ctx.enter_context(tc.tile_pool(name="cpool", bufs=3))
        psum = ctx.enter_context(tc.tile_pool(name="ps", bufs=3, space="PSUM"))
        hpsum = ctx.enter_context(tc.tile_pool(name="hps", bufs=1, space="PSUM"))

        # ---- resident SBUF tensors --------------------------------------
        xt_s = [singles.tile([128, BL, Tn], bf16, tag=f"xt{h}", name=f"xt{h}") for h in range(2)]
        gm1_s = singles.tile([128, Tn, BL], bf16)
        oh_s = singles.tile([128, Tn, BL], bf16)
        hist = singles.tile([128, Tn, BL], bf16)
        w_s = [singles.tile([128, 4 * UNITS], bf16, tag=f"w{h}", name=f"w{h}") for h in range(2)]
        u_s = singles.tile([UNITS, 4 * UNITS], bf16)
        wd_s = singles.tile([UNITS, UNITS], bf16)
        w1_s = singles.tile([UNITS, HID], bf16)
        w2_s = singles.tile([HID, OUT], bf16)
        b1_s = singles.tile([HID, 1], f32)
        b2_s = singles.tile([OUT, 1], f32)
        zero_h = singles.tile([128, BL], bf16)
        zero_cb = singles.tile([128, BL], bf16)
        zero_c = singles.tile([128, BL], f32)
        if slow_bias:
            bias5_s = singles.tile([5, UNITS], f32)
            sel5_s = singles.tile([5, 80], f32)

        # ---- input DMAs --------------------------------------------------
        x3 = xT_d.ap().rearrange("d (b t) -> d b t", b=BL)
        NCH = 1
        ch = Tn // NCH if Tn >= NCH else Tn
        nch = (Tn + ch - 1) // ch
        for h in range(2):
            for ci in range(nch):
                t0, t1 = ci * ch, min((ci + 1) * ch, Tn)
                nc.sync.dma_start(
                    out=xt_s[h][:, :, t0:t1],
                    in_=x3[128 * h : 128 * (h + 1), :, t0:t1],
                )

        g2 = gm1_d.ap()
        o2 = oh_d.ap()
        for ci in range(nch):
            t0, t1 = ci * ch, min((ci + 1) * ch, Tn)
            nc.sync.dma_start(out=gm1_s[:, t0:t1, :], in_=g2[:, t0:t1, :])
            nc.sync.dma_start(out=oh_s[:, t0:t1, :], in_=o2[:, t0:t1, :])
        for h in range(2):
            nc.sync.dma_start(out=w_s[h], in_=Wp_d.ap()[128 * h : 128 * (h + 1), :])
        nc.sync.dma_start(out=u_s, in_=Up_d.ap())
        nc.sync.dma_start(out=wd_s, in_=Wd_d.ap())
        nc.sync.dma_start(out=w1_s, in_=W1_d.ap())
        nc.sync.dma_start(out=w2_s, in_=W2_d.ap())
        nc.sync.dma_start(out=b1_s, in_=b1_d.ap())
        nc.sync.dma_start(out=b2_s, in_=b2_d.ap())
        if slow_bias:
            nc.sync.dma_start(out=bias5_s, in_=bias5_d.ap())
            nc.sync.dma_start(out=sel5_s, in_=sel5_d.ap())

        nc.vector.memset(zero_h, 0.0)
        nc.vector.memset(zero_cb, 0.0)
        nc.vector.memset(zero_c, 0.0)

        # ---- scan --------------------------------------------------------
        # `groups` independent batch column-groups are interleaved per step:
        # their dependency chains are independent, so each group's engine ops
        # execute inside the other's cross-engine latency bubbles.
        GW = BL // groups
        def pre_mms(ps, t, lo, hi):
            if slow_bias:
                nc.tensor.matmul(
                    ps[:, 0:80], bias5_s[:], sel5_s[:], start=True, stop=False
                )
            for g in range(4):
                for h in range(2):
                    nc.tensor.matmul(
                        ps[:, 16 * g : 16 * g + GW],
                        w_s[h][:, 128 * g : 128 * (g + 1)],
                        xt_s[h][:, lo:hi, t],
                        start=(g == 0 and h == 0 and not slow_bias),
                        stop=False,
                    )

        ps_cur = []
        c_cur = []
        h_prev = []
        cbf_prev = []
        for gr in range(groups):
            ps = psum.tile([128, 80], f32, tag=f"ps{gr}", name=f"ps{gr}")
            pre_mms(ps, 0, gr * GW, gr * GW + GW)
            ps_cur.append(ps)
            c_cur.append(zero_c[:, 0:GW])
            h_prev.append(zero_h[:, 0:GW])
            cbf_prev.append(zero_cb[:, 0:GW])

        for t in range(Tn):
            ps_nxt = [None] * groups
            # PE phase: per group CS-MM (cbf from t-1) then U-MMs, then W(t+1)
            for gr in range(groups):
                ps = ps_cur[gr]
                if "no_cs" not in probe:
                    nc.tensor.matmul(
                        ps[:, 64:64 + GW], wd_s[:], cbf_prev[gr][:],
                        start=False, stop=False,
                    )
                for g in range(4):
                    nc.tensor.matmul(
                        ps[:, 16 * g : 16 * g + GW],
                        u_s[:, 128 * g : 128 * (g + 1)],
                        h_prev[gr][:],
                        start=False,
                        stop=(g == 3),
                    )
            for gr in range(groups):
                if t + 1 < Tn:
                    ps_nxt[gr] = psum.tile(
                        [128, 80], f32, tag=f"ps{gr}", name=f"psn{gr}"
                    )
                    pre_mms(ps_nxt[gr], t + 1, gr * GW, gr * GW + GW)
            # ACT: cs+gates per group (emitted group-interleaved so neither
            # group's late ops head-of-line-block the other's ready ops)
            cs = [None] * groups
            S = [None] * groups
            for gr in range(groups):
                if "no_cs" not in probe:
                    cs[gr] = tmp.tile([128, GW], f32, tag=f"cs{gr}", name=f"cs{gr}")
                    nc.scalar.activation(
                        cs[gr], ps_cur[gr][:, 64:64 + GW], AF.Tanh, scale=0.5
                    )
                S[gr] = tmp.tile([128, 4, GW], f32, tag=f"S{gr}", name=f"S{gr}")
                nc.scalar.activation(
                    S[gr],
                    ps_cur[gr][:, 0:64].rearrange("p (g c) -> p g c", g=4)[:, :, 0:GW],
                    AF.Tanh,
                )
            # DVE: c_adj prep both groups, then gate math A then B
            c_adj = [None] * groups
            for gr in range(groups):
                if "no_cs" in probe:
                    c_adj[gr] = c_cur[gr]
                else:
                    q = tmp.tile([128, GW], f32, tag=f"q{gr}", name=f"q{gr}")
                    nc.vector.tensor_mul(
                        q, cs[gr][:], gm1_s[:, t, gr * GW : gr * GW + GW]
                    )
                    c_adj[gr] = tmp.tile([128, GW], f32, tag=f"ca{gr}", name=f"ca{gr}")
                    nc.vector.tensor_add(c_adj[gr], c_cur[gr][:], q[:])
            c_new = [None] * groups
            for gr in range(groups):
                a1 = tmp.tile([128, GW], f32, tag=f"a1{gr}", name=f"a1{gr}")
                nc.vector.scalar_tensor_tensor(
                    a1, S[gr][:, 1, :], 1.0, c_adj[gr][:], OPA.add, OPA.mult
                )
                a2 = tmp.tile([128, GW], f32, tag=f"a2{gr}", name=f"a2{gr}")
                nc.vector.scalar_tensor_tensor(
                    a2, S[gr][:, 0, :], 1.0, S[gr][:, 3, :], OPA.add, OPA.mult
                )
                c_new[gr] = cpool.tile([128, GW], f32, tag=f"cn{gr}", name=f"cn{gr}")
                nc.vector.scalar_tensor_tensor(
                    c_new[gr], a1[:], 0.5, a2[:], OPA.mult, OPA.add
                )
            # ACT: tanh(c) both groups; DVE tail: cbf + h per group
            tc_ = [None] * groups
            for gr in range(groups):
                tc_[gr] = tmp.tile([128, GW], f32, tag=f"tc{gr}", name=f"tc{gr}")
                nc.scalar.activation(tc_[gr], c_new[gr][:], AF.Tanh, scale=0.5)
            for gr in range(groups):
                lo, hi = gr * GW, gr * GW + GW
                if "no_cs" not in probe:
                    cbf = cpool.tile([128, GW], bf16, tag=f"cb{gr}", name=f"cb{gr}")
                    nc.vector.tensor_copy(cbf, c_new[gr][:])
                    cbf_prev[gr] = cbf
                nc.vector.scalar_tensor_tensor(
                    hist[:, t, lo:hi], S[gr][:, 2, :], 1.0, tc_[gr][:],
                    OPA.add, OPA.mult,
                )
                c_cur[gr] = c_new[gr]
                h_prev[gr] = hist[:, t, lo:hi]
            if debug_hist and t == 0:
                dbg_ps = ps_cur[0]
            for gr in range(groups):
                if t + 1 < Tn:
                    ps_cur[gr] = ps_nxt[gr]
            if debug_hist and t == 0:
                s0t = singles.tile([128, 64], f32)
                nc.vector.tensor_copy(s0t, S[0][:].rearrange('p g c -> p (g c)'))
                nc.sync.dma_start(out=s0_d.ap(), in_=s0t[:])
                ps0t = singles.tile([128, 80], f32)
                nc.vector.tensor_copy(ps0t, dbg_ps[:])
                nc.sync.dma_start(out=ps0_d.ap(), in_=ps0t[:])

        # ---- gather at position + head ----------------------------------
        m = singles.tile([128, Tn, BL], bf16)
        nc.vector.tensor_mul(m, hist[:], oh_s[:])
        sel = singles.tile([128, BL], f32)
        nc.vector.tensor_reduce(
            sel, m[:].rearrange("p t b -> p b t"), mybir.AxisListType.X, OPA.add
        )
        selb = singles.tile([128, BL], bf16)
        nc.vector.tensor_copy(selb, sel[:])

        ph1 = hpsum.tile([HID, BL], f32, tag="ph1")
        nc.tensor.matmul(ph1, w1_s[:], selb[:], start=True, stop=True)
        y1 = singles.tile([HID, BL], bf16)
        if sim_gelu:
            # CoreSim lacks Gelu: debug-only x*sigmoid(1.702x) approximation
            y1a = singles.tile([HID, BL], f32)
            nc.scalar.activation(y1a, ph1[:], AF.Copy, bias=0.0)
            nc.vector.tensor_scalar_add(y1a, y1a[:], 0.0)  # keep fp32 copy
            y1b = singles.tile([HID, BL], f32)
            nc.scalar.activation(y1b, ph1[:], AF.Sigmoid, bias=b1_s[:, 0:1], scale=1.702)
            y1c = singles.tile([HID, BL], f32)
            nc.scalar.activation(y1c, ph1[:], AF.Copy, bias=0.0)
            # (x + b1) * sigmoid(1.702(x+b1)): need biased x too
            y1d = singles.tile([HID, BL], f32)
            nc.vector.tensor_scalar(y1d, y1c[:], b1_s[:, 0:1], None, OPA.add)
            nc.vector.tensor_mul(y1, y1d[:], y1b[:])
        else:
            nc.scalar.activation(y1, ph1[:], AF.Gelu, bias=b1_s[:, 0:1])
        ph2 = hpsum.tile([OUT, BL], f32, tag="ph2")
        nc.tensor.matmul(ph2, w2_s[:], y1[:], start=True, stop=True)
        yout = singles.tile([OUT, BL], f32)
        nc.scalar.activation(yout, ph2[:], AF.Sigmoid, bias=b2_s[:, 0:1])
        nc.sync.dma_start(out=out_d.ap(), in_=yout[:])
        if debug_hist:
            xtf = singles.tile([128, BL * Tn], f32)
            nc.vector.tensor_copy(xtf, xt_s[0][:].rearrange("p b t -> p (b t)"))
            nc.sync.dma_start(out=xto_d.ap(), in_=xtf[:])
            wof = singles.tile([128, 512], f32)
            nc.vector.tensor_copy(wof, w_s[0][:])
            nc.sync.dma_start(out=wo_d.ap(), in_=wof[:])
            histf = singles.tile([128, Tn * BL], f32)
            nc.vector.tensor_copy(histf, hist[:].rearrange("p t b -> p (t b)"))
            nc.sync.dma_start(out=histo_d.ap(), in_=histf[:])

    nc.finalize()
    return nc


def prep_inputs(x, time, position, W, U, b, Wd, bd, W1, b1, W2, b2, Tn=T):
    """Host-side prep. Returns (in_maps, slow_bias)."""
    x = np.asarray(x, np.float32)[:, :Tn]
    time = np.asarray(time, np.float32)[:, :Tn]
    position = np.asarray(position).astype(np.int64)
    W = np.asarray(W, np.float32)
    U = np.asarray(U, np.float32)
    b = np.asarray(b, np.float32)
    Wd = np.asarray(Wd, np.float32)
    bd = np.asarray(bd, np.float32)
    W1 = np.asarray(W1, np.float32)
    b1 = np.asarray(b1, np.float32)
    W2 = np.asarray(W2, np.float32)
    b2 = np.asarray(b2, np.float32)

    slow_bias = bool(np.any(b != 0) or np.any(bd != 0))

    # reorder gate columns [i f c o] -> [i f o c], apply all-tanh/state scalings
    def perm(M):
        return np.concatenate([M[:, :256], M[:, 384:], M[:, 256:384]], axis=1)

    Wp = perm(W).copy()
    Wp[:, :384] *= 0.5
    Up = perm(U).copy()
    Up[:, :384] *= 0.25
    Up[:, 384:] *= 0.5
    W1p = W1 * 0.5

    bp = np.concatenate([b[:256], b[384:], b[256:384]])
    bias5 = np.stack(
        [bp[0:128] * 0.5, bp[128:256] * 0.5, bp[256:384] * 0.5, bp[384:512], 2.0 * bd]
    ).astype(np.float32)
    sel5 = np.zeros((5, 80), np.float32)
    for k in range(5):
        sel5[k, 16 * k : 16 * (k + 1)] = 1.0

    gm1_full = (2.0 * (1.0 / np.log(np.e + time) - 1.0)).astype(np.float32)  # [B,Tn]

    common = {
        "Wp": Wp.astype(BF16),
        "Up": Up.astype(BF16),
        "Wdp": Wd.astype(BF16),
        "W1p": W1p.astype(BF16),
        "W2p": W2.astype(BF16),
        "b1v": b1.reshape(HID, 1).astype(np.float32),
        "b2v": b2.reshape(OUT, 1).astype(np.float32),
    }
    if slow_bias:
        common["bias5"] = bias5
        common["sel5"] = sel5

    in_maps = []
    for k in range(NCORES):
        sl = slice(BL * k, BL * (k + 1))
        xT = (
            np.ascontiguousarray(x[sl].transpose(2, 0, 1))
            .reshape(D, BL * Tn)
            .astype(BF16)
        )
        gm1 = np.broadcast_to(
            np.ascontiguousarray(gm1_full[sl].T).astype(BF16), (128, Tn, BL)
        ).copy()
        oh = np.zeros((Tn, BL), np.float32)
        for bb in range(BL):
            p = min(int(position[BL * k + bb]), Tn - 1)
            oh[p, bb] = 1.0
        im = dict(common)
        im["xT"] = xT
        im["gm1"] = gm1
        im["oh"] = np.broadcast_to(oh.astype(BF16), (128, Tn, BL)).copy()
        in_maps.append(im)
    return in_maps, slow_bias


_CACHE = {}


def run(inputs, Tn=T, trace=False):
    from concourse.bass_utils import run_bass_kernel_spmd

    in_maps, slow_bias = prep_inputs(**inputs, Tn=Tn)
    key = (Tn, slow_bias)
    if key not in _CACHE:
        _CACHE[key] = build_module(Tn, slow_bias)
    nc = _CACHE[key]
    res = run_bass_kernel_spmd(
        nc, in_maps, core_ids=list(range(NCORES)), trace=trace
    )
    out = np.zeros((B, OUT), np.float32)
    for k in range(NCORES):
        out[BL * k : BL * (k + 1)] = np.asarray(
            res.results[k]["outT"], np.float32
        ).T
    return out, res


def kernel(**inputs) -> np.ndarray:
    out, _ = run(inputs, Tn=T, trace=False)
    return out
